# revision 17
# baseline (speedup 1.0000x reference)
"""Trainium2 Bass kernel for nn_BinaryPathEncoder.

Math: for each position p, the ordered product of rotation matrices along
p's binary path (LSB-first, leading 1-bit stripped):
    R(p) = M_{b0} @ M_{b1} @ ... @ M_{b(k-1)},  M_b = expm(B_b - B_b^T)^T
Key identity: R(p) = M_{p&1} @ R(p>>1).  Splitting the <=16-step path into
6+5+5 bit chunks gives R(p) = R(idxA) @ R(idxB) @ R(idxC) with idxA<128,
idxB<64, idxC<64, so two small fp16 SBUF tables (natural R[q], q<64, and
transposed R[q]^T, q in {1} u [64,128)) cover every position with 2 matmuls:
  product1: X1T = matmul(lhsT=Rn[idxB](staged), rhs=Rt[idxA]) = (TA@TB)^T
  product2: O   = matmul(lhsT=X1T,              rhs=Rn[idxC]) = TA@TB@TC

Performance structure (all data-independent work on the host):
  - expm + the 128 table entries are computed on host (numpy f64) and
    uploaded as f16 inputs; no on-device expm/table-build phase.
  - tables are stored KC-SPLIT (separate SBUF tensors for contraction rows
    0..127 and 128..255) so the two contraction matmuls of a product share
    ONE register offset value: walrus emits one InstFusedRegOps per value
    switch, so kc-split + kc-outer ordering costs 2 fused-ops per position
    instead of 8.
  - the per-position stationary operand stream (R[idxB]) is assembled on
    host into a DRAM tensor and staged into SBUF with batched static DMAs
    (4 positions per DMA from gpsimd).
  - moving-operand offsets load 4 registers per TWO pipeline steps with a
    single TENSOR_LOAD, snapped to immutable values.
  - positions are sorted by idxC per core so the rn table loads in chunks
    (per-chunk semaphores) overlapped with the position pipeline; output
    rows are unpermuted on the host.
  - output DMAs are batched 2 positions per issue; DMA completions are not
    ordered across issues, so slot-recycling waits use rotating per-group
    semaphores.
"""

import contextlib
import numpy as np

DIM = 256
NCORES = 8
P = 128

NAT_E = 63                     # natural table entries (q in [1,64))
TRA_E = 65                     # transposed entries: slot0=identity, 1..64 = q in [64,128)
ENT = 512                      # elements per partition per entry (2 kc x 256)
HENT = 256                     # elements per partition per entry per kc-half

SBATCH = 4                     # positions per staging DMA
NSTG = 16                      # stag_b slots (4 batches in flight)
NSTG_SEM = 4                   # rotating staging sems
NSX = 12                       # X1T staging slots
NOUT = 16                      # output buffer slots
NDMA_SEM = 8                   # rotating out-dma sems
CLAG = 3                       # mm2 pipeline lag (positions)
RN_CHUNKS = 8                  # rn table load chunks (overlap with compute)
RT_CHUNKS = 4                  # rt table load chunks (per-core first-use order)

_NC_CACHE = {}
LAST_RESULTS = None


def _build_nc(npos, rn_need, rt_need, debug=False):
    """rn_need[q] / rt_need[s] = number of rn/rt chunks that must be
    resident before mm2 step q / mm1 step s executes; nondecreasing,
    computed from the actual (sorted, first-use-permuted) per-core index
    data, max'd across cores."""
    from concourse import bass, bacc, mybir

    f32 = mybir.dt.float32
    f16 = mybir.dt.float16
    i32 = mybir.dt.int32

    assert npos % SBATCH == 0 and npos % 2 == 0
    nsteps = npos + CLAG
    ngrp = (nsteps + 3) // 4
    ncolw = (ngrp + P - 1) // P            # offset word groups per partition

    nc = bacc.Bacc("TRN2", target_bir_lowering=False, debug=debug)

    rt0_ext = nc.dram_tensor("rt0", [P, TRA_E * HENT], f16, kind="ExternalInput")
    rt1_ext = nc.dram_tensor("rt1", [P, TRA_E * HENT], f16, kind="ExternalInput")
    rn0_ext = nc.dram_tensor("rn0", [P, NAT_E * HENT], f16, kind="ExternalInput")
    rn1_ext = nc.dram_tensor("rn1", [P, NAT_E * HENT], f16, kind="ExternalInput")
    bs_ext = nc.dram_tensor("bstream", [P, npos * ENT], f16, kind="ExternalInput")
    offs_ext = nc.dram_tensor("offs", [P, 8 * ncolw], i32, kind="ExternalInput")
    out_ext = nc.dram_tensor("out", [npos, 2, P, DIM], f16, kind="ExternalOutput")

    # rn chunk boundaries (slots 0..62 -> chunk k covers slots [ch[k],ch[k+1]))
    ch = [(k * NAT_E) // RN_CHUNKS for k in range(RN_CHUNKS + 1)]
    # rt chunk boundaries over the 65 first-use-permuted ranks
    cht = [(k * TRA_E) // RT_CHUNKS for k in range(RT_CHUNKS + 1)]

    with contextlib.ExitStack() as ctx:
        sem = {}
        for name in (["offs_sem", "mm1_sem", "mm2_sem",
                      "dvex_sem", "act_sem"]
                     + [f"rt_c{k}" for k in range(RT_CHUNKS)]
                     + [f"rn_c{k}" for k in range(RN_CHUNKS)]
                     + [f"stg_s{j}" for j in range(NSTG_SEM)]
                     + [f"dma_s{j}" for j in range(NDMA_SEM)]):
            sem[name] = ctx.enter_context(nc.semaphore(name))

        rt0 = ctx.enter_context(nc.sbuf_tensor("rt0_sb", [P, TRA_E * HENT], f16))
        rt1 = ctx.enter_context(nc.sbuf_tensor("rt1_sb", [P, TRA_E * HENT], f16))
        rn0 = ctx.enter_context(nc.sbuf_tensor("rn0_sb", [P, NAT_E * HENT], f16))
        rn1 = ctx.enter_context(nc.sbuf_tensor("rn1_sb", [P, NAT_E * HENT], f16))
        offs = ctx.enter_context(nc.sbuf_tensor("offs_sb", [P, 8 * ncolw], i32))
        stag_b = ctx.enter_context(nc.sbuf_tensor("stag_b", [P, NSTG, 2, DIM], f16))
        stag_x = ctx.enter_context(nc.sbuf_tensor("stag_x", [P, NSX, 2, DIM], f16))
        outb = ctx.enter_context(nc.sbuf_tensor("outb", [P, NOUT, 2, DIM], f16))
        ps = [ctx.enter_context(nc.psum_tensor(f"ps{j}", [P, 2, DIM], f32))
              for j in range(8)]

        TSTR = TRA_E * HENT
        NSTR = NAT_E * HENT

        # ---------------- programs ----------------
        # Queue discipline (HWDGE rings are FIFO per engine; completions
        # unordered across rings): SP carries offs + out-DMAs only; ACT
        # carries the rt chunks (issued before its copy loop, transfers
        # naturally prioritized); GPSIMD (SWDGE) carries staging + the rn
        # chunks spread between staging batches.
        def s_in(s):
            s.dma_start(offs[:, :], offs_ext[:, :]).then_inc(sem["offs_sem"], 16)

        def a_in(a):
            for k in range(RT_CHUNKS):
                lo, hi = cht[k] * HENT, cht[k + 1] * HENT
                a.dma_start(rt0[:, lo:hi], rt0_ext[:, lo:hi],
                            ).then_inc(sem[f"rt_c{k}"], 16)
                a.dma_start(rt1[:, lo:hi], rt1_ext[:, lo:hi],
                            ).then_inc(sem[f"rt_c{k}"], 16)

        def g_stage(g):
            rn_k = 0

            def issue_rn(g, k):
                lo, hi = ch[k] * HENT, ch[k + 1] * HENT
                g.dma_start(rn0[:, lo:hi], rn0_ext[:, lo:hi],
                            ).then_inc(sem[f"rn_c{k}"], 16)
                g.dma_start(rn1[:, lo:hi], rn1_ext[:, lo:hi],
                            ).then_inc(sem[f"rn_c{k}"], 16)

            issue_rn(g, 0)
            rn_k = 1
            for b in range(npos // SBATCH):
                if b >= NSTG // SBATCH:
                    g.wait_ge(sem["mm1_sem"], SBATCH * b - (NSTG - SBATCH))
                slot = (SBATCH * b) % NSTG
                g.dma_start(stag_b[:, slot:slot + SBATCH, :, :],
                            bs_ext[:, SBATCH * b * ENT:SBATCH * (b + 1) * ENT],
                            ).then_inc(sem[f"stg_s{b % NSTG_SEM}"], 16)
                if b % 4 == 3 and rn_k < RN_CHUNKS:
                    issue_rn(g, rn_k)
                    rn_k += 1
            while rn_k < RN_CHUNKS:
                issue_rn(g, rn_k)
                rn_k += 1

        def p_pos(t):
            t.wait_ge(sem["offs_sem"], 16)
            regs = [ctx.enter_context(t.register(f"rp{j}")) for j in range(8)]
            vals = {}                       # step -> (vA, vC)

            def tl(grp):
                """load+snap values for steps 4*grp .. 4*grp+3."""
                if 4 * grp >= nsteps:
                    return
                pi, wi = grp % P, 8 * (grp // P)
                t.reg_load(regs, offs[pi:pi + 1, wi:wi + 8])
                for h in range(4):
                    s_ = 4 * grp + h
                    if s_ >= nsteps:
                        continue
                    vA = t.snap(regs[2 * h], donate=False,
                                min_val=0, max_val=(TRA_E - 1) * HENT)
                    vC = t.snap(regs[2 * h + 1], donate=False,
                                min_val=0, max_val=(NAT_E - 1) * HENT)
                    vals[s_] = (vA, vC)

            def mm1(s):
                # mc-outer: PSUM accumulation groups must not interleave
                # within a bank (HW); the kc-split tables share one snapped
                # value so this still costs a single fused-regop per product.
                vA = vals[s][0]
                last = None
                for mc in range(2):
                    for kc in range(2):
                        rhs = bass.AP(rt0 if kc == 0 else rt1, vA,
                                      [[TSTR, P], [1, DIM]])
                        last = t.matmul(ps[s % 4][:, mc, :],
                                        stag_b[:, s % NSTG, kc, mc * P:(mc + 1) * P],
                                        rhs, start=(kc == 0), stop=(kc == 1),
                                        skip_group_check=True)
                last.then_inc(sem["mm1_sem"], 1)

            def mm2(q, s):
                vC = vals[s][1]
                last = None
                for mc in range(2):
                    for kc in range(2):
                        rhs = bass.AP(rn0 if kc == 0 else rn1, vC,
                                      [[NSTR, P], [1, DIM]])
                        last = t.matmul(ps[4 + q % 4][:, mc, :],
                                        stag_x[:, q % NSX, kc, mc * P:(mc + 1) * P],
                                        rhs, start=(kc == 0), stop=(kc == 1),
                                        skip_group_check=True)
                last.then_inc(sem["mm2_sem"], 1)

            tl(0)                           # prologue: steps 0, 1
            rn_have = 0
            rt_have = 0
            for s in range(nsteps):
                if s < npos:
                    need_t = rt_need[s]
                    while rt_have < need_t:
                        t.wait_ge(sem[f"rt_c{rt_have}"], 32)
                        rt_have += 1
                    if s % SBATCH == 0:
                        b = s // SBATCH
                        t.wait_ge(sem[f"stg_s{b % NSTG_SEM}"],
                                  16 * (b // NSTG_SEM + 1))
                    if s >= 4:
                        t.wait_ge(sem["dvex_sem"], s - 3)
                    mm1(s)
                if s % 4 == 3:
                    tl((s + 1) // 4)        # values for steps s+1 .. s+4
                q = s - CLAG
                if 0 <= q < npos:
                    need = rn_need[q]
                    while rn_have < need:
                        t.wait_ge(sem[f"rn_c{rn_have}"], 32)
                        rn_have += 1
                    t.wait_ge(sem["dvex_sem"], q + 1)
                    if q >= 4:
                        t.wait_ge(sem["act_sem"], q - 3)
                    mm2(q, s)
                vals.pop(s, None)

        def d_pos(d):
            for i in range(npos):
                d.wait_ge(sem["mm1_sem"], i + 1)
                if i >= NSX:
                    d.wait_ge(sem["mm2_sem"], i - NSX + 1)
                d.tensor_copy(stag_x[:, i % NSX, :, :],
                              ps[i % 4][:, :, :]).then_inc(sem["dvex_sem"], 1)

        def a_pos(a):
            for q in range(npos):
                a.wait_ge(sem["mm2_sem"], q + 1)
                if q >= NOUT:
                    dd = (q - NOUT) // 2    # the dma that freed this slot
                    a.wait_ge(sem[f"dma_s{dd % NDMA_SEM}"],
                              16 * (dd // NDMA_SEM + 1))
                a.mul(outb[:, q % NOUT, :, :],
                      ps[4 + q % 4][:, :, :], 1.0).then_inc(sem["act_sem"], 1)

        def s_out(s):
            for d_i in range(npos // 2):
                s.wait_ge(sem["act_sem"], 2 * d_i + 2)
                o = (2 * d_i) % NOUT
                dst = bass.AP(out_ext, 2 * d_i * 2 * P * DIM,
                              [[DIM, P], [2 * P * DIM, 2], [P * DIM, 2], [1, DIM]])
                s.dma_start(dst, outb[:, o:o + 2, :, :],
                            ).then_inc(sem[f"dma_s{d_i % NDMA_SEM}"], 16)
            for j in range(NDMA_SEM):
                uses = len([d for d in range(npos // 2) if d % NDMA_SEM == j])
                s.wait_ge(sem[f"dma_s{j}"], 16 * uses)
            for j in range(NSTG_SEM):
                uses = len([b for b in range(npos // SBATCH)
                            if b % NSTG_SEM == j])
                s.wait_ge(sem[f"stg_s{j}"], 16 * uses)

        with nc.Block() as block:
            @block.tensor
            def _(tensor):
                p_pos(tensor)

            @block.vector
            def _(vector):
                d_pos(vector)

            @block.scalar
            def _(scalar):
                a_in(scalar)
                a_pos(scalar)

            @block.gpsimd
            def _(gpsimd):
                g_stage(gpsimd)

            @block.sync
            def _(sync):
                s_in(sync)
                s_out(sync)

    return nc


def _expm(a):
    """expm via scaling-and-squaring Taylor in f64."""
    a = np.asarray(a, dtype=np.float64)
    nrm = np.linalg.norm(a, ord=1)
    s = max(0, int(np.ceil(np.log2(max(nrm, 1e-30) / 0.25))))
    a = a / (2.0 ** s)
    n = a.shape[0]
    e = np.eye(n)
    term = np.eye(n)
    for k in range(1, 20):
        term = term @ a / k
        e = e + term
        if np.abs(term).max() < 1e-18:
            break
    for _ in range(s):
        e = e @ e
    return e


def _tables(primitives):
    """R[q] (f64) for q in [1,128): R(1)=I, R(q) = M_{q&1} @ R(q>>1)."""
    prims = np.asarray(primitives, dtype=np.float64)
    mats = []
    for b in range(2):
        skew = prims[b] - prims[b].T
        mats.append(_expm(skew).T)         # M_b = P_b^T
    R = [None] * 128
    R[1] = np.eye(DIM)
    for q in range(2, 128):
        R[q] = mats[q & 1] @ R[q >> 1]
    return R


def _indices(u):
    """u: (n,) positions -> idxA (transposed tab), idxB, idxC (natural)."""
    u = u.astype(np.int64)
    blen = np.zeros_like(u)
    t = u.copy()
    while np.any(t > 0):
        blen = np.where(t > 0, blen + 1, blen)
        t >>= 1
    k = blen - 1
    tA = np.minimum(k, 6)
    idxA = (1 << tA) + (u & ((1 << tA) - 1))
    tB = np.clip(k - 6, 0, 5)
    idxB = (1 << tB) + ((u >> 6) & ((1 << tB) - 1))
    tC = np.clip(k - 11, 0, 5)
    idxC = (1 << tC) + ((u >> 11) & ((1 << tC) - 1))
    short = u < 64
    idxA = np.where(short, 1, idxA)
    idxB = np.where(short, u, idxB)
    assert idxA.max() < 128 and idxB.max() < 64 and idxC.max() < 64
    assert np.all((idxA == 1) | (idxA >= 64))
    return idxA, idxB, idxC


def kernel(primitives, identity, unique):
    global LAST_RESULTS
    from concourse.bass_utils import run_bass_kernel_spmd

    u = np.asarray(unique).astype(np.int64).ravel()
    n = u.shape[0]
    assert n % NCORES == 0
    npos = n // NCORES
    nsteps = npos + CLAG
    ngrp = (nsteps + 3) // 4
    ncolw = (ngrp + P - 1) // P

    R = _tables(primitives)
    # kc-split per-partition layouts: half h of entry M is M[h*128+p, j]
    rn0_np = np.zeros((P, NAT_E * HENT), np.float16)
    rn1_np = np.zeros((P, NAT_E * HENT), np.float16)
    for q in range(1, 64):
        m = R[q].astype(np.float16)
        rn0_np[:, (q - 1) * HENT:q * HENT] = m[0:P, :]
        rn1_np[:, (q - 1) * HENT:q * HENT] = m[P:2 * P, :]

    idxA, idxB, idxC = _indices(u)
    slotA = np.where(idxA == 1, 0, idxA - 63)
    vC_all = ((idxC - 1) * HENT).astype(np.int32)

    # rn chunk boundaries must match _build_nc
    ch = [(k * NAT_E) // RN_CHUNKS for k in range(RN_CHUNKS + 1)]
    cht = [(k * TRA_E) // RT_CHUNKS for k in range(RT_CHUNKS + 1)]
    slotC = (idxC - 1).astype(np.int64)
    chunk_of = np.zeros(n, np.int64)
    for k in range(RN_CHUNKS):
        chunk_of[(slotC >= ch[k]) & (slotC < ch[k + 1])] = k

    # per-core sort by rn chunk; rn_need[q] = chunks resident before mm2
    # step q, max'd across cores (one SPMD program must fit all cores).
    # rt is stored per-core in FIRST-USE rank order so it can be loaded in
    # chunks gated by rt_need[s] while the pipeline runs.
    perms = []
    sorted_chunks = []
    rank_of_core = []           # per core: slotA -> rank (first-use order)
    rank_seq = []               # per core: per sorted step, the rank used
    for c in range(NCORES):
        sl = slice(c * npos, (c + 1) * npos)
        order = np.argsort(chunk_of[sl], kind="stable")
        perms.append(order)
        sorted_chunks.append(chunk_of[sl][order])
        sA = slotA[sl][order]
        rank = {}
        seq = np.zeros(npos, np.int64)
        for s in range(npos):
            v = int(sA[s])
            if v not in rank:
                rank[v] = len(rank)
            seq[s] = rank[v]
        for v in range(TRA_E):          # unused entries fill the tail ranks
            if v not in rank:
                rank[v] = len(rank)
        rank_of_core.append(rank)
        rank_seq.append(seq)
    rn_need = (np.max(np.stack(sorted_chunks), axis=0) + 1).astype(np.int64)
    rn_need = np.maximum.accumulate(rn_need)

    max_rank = np.maximum.accumulate(np.max(np.stack(rank_seq), axis=0))
    chunk_of_rank = np.zeros(TRA_E, np.int64)
    for k in range(RT_CHUNKS):
        chunk_of_rank[cht[k]:cht[k + 1]] = k
    rt_need = chunk_of_rank[max_rank] + 1
    key = (npos, tuple(rn_need.tolist()), tuple(rt_need.tolist()))
    if key not in _NC_CACHE:
        nc = _build_nc(npos, rn_need.tolist(), rt_need.tolist())
        nc.compile()
        _NC_CACHE[key] = nc
    nc = _NC_CACHE[key]

    in_maps = []
    for c in range(NCORES):
        sl = slice(c * npos, (c + 1) * npos)
        order = perms[c]
        iB = idxB[sl][order]
        a_v = (rank_seq[c] * HENT).astype(np.int32)
        c_v = vC_all[sl][order]

        # per-core rt tables in first-use rank order
        rt0_c = np.zeros((P, TRA_E * HENT), np.float16)
        rt1_c = np.zeros((P, TRA_E * HENT), np.float16)
        slot_to_q = {0: 1}
        for q in range(64, 128):
            slot_to_q[q - 63] = q
        for v, r in rank_of_core[c].items():
            m = R[slot_to_q[v]].T.astype(np.float16)
            rt0_c[:, r * HENT:(r + 1) * HENT] = m[0:P, :]
            rt1_c[:, r * HENT:(r + 1) * HENT] = m[P:2 * P, :]

        # bstream: entry idxB[k] in full [p, kc*256+j] layout, partition-major
        bstream = np.zeros((P, npos * ENT), np.float16)
        for k_i in range(npos):
            q = iB[k_i]
            m = R[q].astype(np.float16)
            bstream[:, k_i * ENT:k_i * ENT + HENT] = m[0:P, :]
            bstream[:, k_i * ENT + HENT:(k_i + 1) * ENT] = m[P:2 * P, :]

        offs_np = np.zeros((P, 8 * ncolw), np.int32)
        for g in range(ngrp):
            w = np.zeros(8, np.int32)
            for h in range(4):
                s = 4 * g + h
                if s < npos:
                    w[2 * h] = a_v[s]
                q = s - CLAG
                if 0 <= q < npos:
                    w[2 * h + 1] = c_v[q]
            offs_np[g % P, 8 * (g // P):8 * (g // P) + 8] = w

        in_maps.append({"rt0": rt0_c, "rt1": rt1_c, "rn0": rn0_np,
                        "rn1": rn1_np, "bstream": bstream, "offs": offs_np})

    import os
    trace_dir = os.environ.get("KERNEL_TRACE_DIR")
    res = run_bass_kernel_spmd(nc, in_maps, core_ids=list(range(NCORES)),
                               tmpdir=trace_dir)
    LAST_RESULTS = res

    out = np.empty((n, DIM, DIM), np.float32)
    for c in range(NCORES):
        o = np.asarray(res.results[c]["out"])  # (npos, 2, 128, 256) f16
        o = o.reshape(npos, DIM, DIM).astype(np.float32)
        out[c * npos + perms[c]] = o

    ident = np.asarray(identity, dtype=np.float32)[0]
    if not np.allclose(ident, np.eye(DIM, dtype=np.float32)):
        out = np.einsum("ij,njk->nik", ident, out).astype(np.float32)
    return out


# revision 18
# speedup vs baseline: 1.1135x; 1.1135x over previous
"""Trainium2 Bass kernel for nn_BinaryPathEncoder.

Math: for each position p, the ordered product of rotation matrices along
p's binary path (LSB-first, leading 1-bit stripped):
    R(p) = M_{b0} @ M_{b1} @ ... @ M_{b(k-1)},  M_b = expm(B_b - B_b^T)^T
Key identity: R(p) = M_{p&1} @ R(p>>1).  Splitting the <=16-step path into
6+5+5 bit chunks gives R(p) = R(idxA) @ R(idxB) @ R(idxC) with idxA<128,
idxB<64, idxC<64, so two small fp16 SBUF tables (natural R[q], q<64, and
transposed R[q]^T, q in {1} u [64,128)) cover every position with 2 matmuls:
  product1: X1T = matmul(lhsT=Rn[idxB](staged), rhs=Rt[idxA]) = (TA@TB)^T
  product2: O   = matmul(lhsT=X1T,              rhs=Rn[idxC]) = TA@TB@TC

Performance structure (all data-independent work on the host):
  - expm + the 128 table entries are computed on host (numpy f64) and
    uploaded as f16 inputs; no on-device expm/table-build phase.
  - tables are stored KC-SPLIT (separate SBUF tensors for contraction rows
    0..127 and 128..255) so the two contraction matmuls of a product share
    ONE register offset value: walrus emits one InstFusedRegOps per value
    switch, so kc-split + kc-outer ordering costs 2 fused-ops per position
    instead of 8.
  - the per-position stationary operand stream (R[idxB]) is assembled on
    host into a DRAM tensor and staged into SBUF with batched static DMAs
    (4 positions per DMA from gpsimd).
  - moving-operand offsets load 4 registers per TWO pipeline steps with a
    single TENSOR_LOAD, snapped to immutable values.
  - positions are sorted by idxC per core so the rn table loads in chunks
    (per-chunk semaphores) overlapped with the position pipeline; output
    rows are unpermuted on the host.
  - output DMAs are batched 2 positions per issue; DMA completions are not
    ordered across issues, so slot-recycling waits use rotating per-group
    semaphores.
"""

import contextlib
import numpy as np

DIM = 256
NCORES = 8
P = 128

NAT_E = 63                     # natural table entries (q in [1,64))
TRA_E = 65                     # transposed entries: slot0=identity, 1..64 = q in [64,128)
ENT = 512                      # elements per partition per entry (2 kc x 256)
HENT = 256                     # elements per partition per entry per kc-half

SBATCH = 4                     # positions per staging DMA
NSTG = 16                      # stag_b slots (4 batches in flight)
NSTG_SEM = 4                   # rotating staging sems
NSX = 12                       # X1T staging slots
NOUT = 16                      # output buffer slots
NDMA_SEM = 8                   # rotating out-dma sems
CLAG = 3                       # mm2 pipeline lag (positions)
RN_CHUNKS = 8                  # rn table load chunks (overlap with compute)
RT_CHUNKS = 4                  # rt table load chunks (per-core first-use order)
RT_CH_BOUNDS = [0, 4, 20, 40, 65]  # tiny first chunk: ranks grow <=1/step

_NC_CACHE = {}
LAST_RESULTS = None


def _build_nc(npos, rn_need, rt_need, debug=False):
    """rn_need[q] / rt_need[s] = number of rn/rt chunks that must be
    resident before mm2 step q / mm1 step s executes; nondecreasing,
    computed from the actual (sorted, first-use-permuted) per-core index
    data, max'd across cores."""
    from concourse import bass, bacc, mybir

    f32 = mybir.dt.float32
    f16 = mybir.dt.float16
    i32 = mybir.dt.int32

    assert npos % SBATCH == 0 and npos % 2 == 0
    nsteps = npos + CLAG
    ngrp = (nsteps + 3) // 4
    ncolw = (ngrp + P - 1) // P            # offset word groups per partition

    nc = bacc.Bacc("TRN2", target_bir_lowering=False, debug=debug)

    rt0_ext = nc.dram_tensor("rt0", [P, TRA_E * HENT], f16, kind="ExternalInput")
    rt1_ext = nc.dram_tensor("rt1", [P, TRA_E * HENT], f16, kind="ExternalInput")
    rn0_ext = nc.dram_tensor("rn0", [P, NAT_E * HENT], f16, kind="ExternalInput")
    rn1_ext = nc.dram_tensor("rn1", [P, NAT_E * HENT], f16, kind="ExternalInput")
    bs_ext = nc.dram_tensor("bstream", [P, npos * ENT], f16, kind="ExternalInput")
    offs_ext = nc.dram_tensor("offs", [P, 8 * ncolw], i32, kind="ExternalInput")
    out_ext = nc.dram_tensor("out", [npos, 2, P, DIM], f16, kind="ExternalOutput")

    # rn chunk boundaries (slots 0..62 -> chunk k covers slots [ch[k],ch[k+1]))
    ch = [(k * NAT_E) // RN_CHUNKS for k in range(RN_CHUNKS + 1)]
    # rt chunk boundaries over the 65 first-use-permuted ranks
    cht = RT_CH_BOUNDS

    with contextlib.ExitStack() as ctx:
        sem = {}
        for name in (["offs_sem", "mm1_sem", "mm2_sem",
                      "dvex_sem", "act_sem"]
                     + [f"rt_c{k}" for k in range(RT_CHUNKS)]
                     + [f"rn_c{k}" for k in range(RN_CHUNKS)]
                     + [f"stg_s{j}" for j in range(NSTG_SEM)]
                     + [f"dma_s{j}" for j in range(NDMA_SEM)]):
            sem[name] = ctx.enter_context(nc.semaphore(name))

        rt0 = ctx.enter_context(nc.sbuf_tensor("rt0_sb", [P, TRA_E * HENT], f16))
        rt1 = ctx.enter_context(nc.sbuf_tensor("rt1_sb", [P, TRA_E * HENT], f16))
        rn0 = ctx.enter_context(nc.sbuf_tensor("rn0_sb", [P, NAT_E * HENT], f16))
        rn1 = ctx.enter_context(nc.sbuf_tensor("rn1_sb", [P, NAT_E * HENT], f16))
        offs = ctx.enter_context(nc.sbuf_tensor("offs_sb", [P, 8 * ncolw], i32))
        stag_b = ctx.enter_context(nc.sbuf_tensor("stag_b", [P, NSTG, 2, DIM], f16))
        stag_x = ctx.enter_context(nc.sbuf_tensor("stag_x", [P, NSX, 2, DIM], f16))
        outb = ctx.enter_context(nc.sbuf_tensor("outb", [P, NOUT, 2, DIM], f16))
        ps = [ctx.enter_context(nc.psum_tensor(f"ps{j}", [P, 2, DIM], f32))
              for j in range(8)]

        TSTR = TRA_E * HENT
        NSTR = NAT_E * HENT

        # ---------------- programs ----------------
        # Queue discipline (HWDGE rings are FIFO per engine; completions
        # unordered across rings): SP carries offs + out-DMAs only; ACT
        # carries the rt chunks (issued before its copy loop, transfers
        # naturally prioritized); GPSIMD (SWDGE) carries staging + the rn
        # chunks spread between staging batches.
        def s_in(s):
            s.dma_start(offs[:, :], offs_ext[:, :]).then_inc(sem["offs_sem"], 16)

        def a_in(a):
            for k in range(RT_CHUNKS):
                lo, hi = cht[k] * HENT, cht[k + 1] * HENT
                a.dma_start(rt0[:, lo:hi], rt0_ext[:, lo:hi],
                            ).then_inc(sem[f"rt_c{k}"], 16)
                a.dma_start(rt1[:, lo:hi], rt1_ext[:, lo:hi],
                            ).then_inc(sem[f"rt_c{k}"], 16)

        def g_stage(g):
            rn_k = 0

            def issue_rn(g, k):
                lo, hi = ch[k] * HENT, ch[k + 1] * HENT
                g.dma_start(rn0[:, lo:hi], rn0_ext[:, lo:hi],
                            ).then_inc(sem[f"rn_c{k}"], 16)
                g.dma_start(rn1[:, lo:hi], rn1_ext[:, lo:hi],
                            ).then_inc(sem[f"rn_c{k}"], 16)

            rn_k = 0
            for b in range(npos // SBATCH):
                if b >= NSTG // SBATCH:
                    g.wait_ge(sem["mm1_sem"], SBATCH * b - (NSTG - SBATCH))
                slot = (SBATCH * b) % NSTG
                g.dma_start(stag_b[:, slot:slot + SBATCH, :, :],
                            bs_ext[:, SBATCH * b * ENT:SBATCH * (b + 1) * ENT],
                            ).then_inc(sem[f"stg_s{b % NSTG_SEM}"], 16)
                if b % 4 == 0 and rn_k < RN_CHUNKS:
                    issue_rn(g, rn_k)
                    rn_k += 1
            while rn_k < RN_CHUNKS:
                issue_rn(g, rn_k)
                rn_k += 1

        def p_pos(t):
            t.wait_ge(sem["offs_sem"], 16)
            regs = [ctx.enter_context(t.register(f"rp{j}")) for j in range(8)]
            vals = {}                       # step -> (vA, vC)

            def tl(grp):
                """load+snap values for steps 4*grp .. 4*grp+3."""
                if 4 * grp >= nsteps:
                    return
                pi, wi = grp % P, 8 * (grp // P)
                t.reg_load(regs, offs[pi:pi + 1, wi:wi + 8])
                for h in range(4):
                    s_ = 4 * grp + h
                    if s_ >= nsteps:
                        continue
                    vA = t.snap(regs[2 * h], donate=False,
                                min_val=0, max_val=(TRA_E - 1) * HENT)
                    vC = t.snap(regs[2 * h + 1], donate=False,
                                min_val=0, max_val=(NAT_E - 1) * HENT)
                    vals[s_] = (vA, vC)

            def mm1(s):
                # mc-outer: PSUM accumulation groups must not interleave
                # within a bank (HW); the kc-split tables share one snapped
                # value so this still costs a single fused-regop per product.
                vA = vals[s][0]
                last = None
                for mc in range(2):
                    for kc in range(2):
                        rhs = bass.AP(rt0 if kc == 0 else rt1, vA,
                                      [[TSTR, P], [1, DIM]])
                        last = t.matmul(ps[s % 4][:, mc, :],
                                        stag_b[:, s % NSTG, kc, mc * P:(mc + 1) * P],
                                        rhs, start=(kc == 0), stop=(kc == 1),
                                        skip_group_check=True)
                last.then_inc(sem["mm1_sem"], 1)

            def mm2(q, s):
                vC = vals[s][1]
                last = None
                for mc in range(2):
                    for kc in range(2):
                        rhs = bass.AP(rn0 if kc == 0 else rn1, vC,
                                      [[NSTR, P], [1, DIM]])
                        last = t.matmul(ps[4 + q % 4][:, mc, :],
                                        stag_x[:, q % NSX, kc, mc * P:(mc + 1) * P],
                                        rhs, start=(kc == 0), stop=(kc == 1),
                                        skip_group_check=True)
                last.then_inc(sem["mm2_sem"], 1)

            tl(0)                           # prologue: steps 0, 1
            rn_have = 0
            rt_have = 0
            for s in range(nsteps):
                if s < npos:
                    need_t = rt_need[s]
                    while rt_have < need_t:
                        t.wait_ge(sem[f"rt_c{rt_have}"], 32)
                        rt_have += 1
                    if s % SBATCH == 0:
                        b = s // SBATCH
                        t.wait_ge(sem[f"stg_s{b % NSTG_SEM}"],
                                  16 * (b // NSTG_SEM + 1))
                    if s >= 4:
                        t.wait_ge(sem["dvex_sem"], s - 3)
                    mm1(s)
                if s % 4 == 3:
                    tl((s + 1) // 4)        # values for steps s+1 .. s+4
                q = s - CLAG
                if 0 <= q < npos:
                    need = rn_need[q]
                    while rn_have < need:
                        t.wait_ge(sem[f"rn_c{rn_have}"], 32)
                        rn_have += 1
                    t.wait_ge(sem["dvex_sem"], q + 1)
                    if q >= 4:
                        t.wait_ge(sem["act_sem"], q - 3)
                    mm2(q, s)
                vals.pop(s, None)

        def d_pos(d):
            for i in range(npos):
                d.wait_ge(sem["mm1_sem"], i + 1)
                if i >= NSX:
                    d.wait_ge(sem["mm2_sem"], i - NSX + 1)
                d.tensor_copy(stag_x[:, i % NSX, :, :],
                              ps[i % 4][:, :, :]).then_inc(sem["dvex_sem"], 1)

        def a_pos(a):
            for q in range(npos):
                a.wait_ge(sem["mm2_sem"], q + 1)
                if q >= NOUT:
                    dd = (q - NOUT) // 2    # the dma that freed this slot
                    a.wait_ge(sem[f"dma_s{dd % NDMA_SEM}"],
                              16 * (dd // NDMA_SEM + 1))
                a.mul(outb[:, q % NOUT, :, :],
                      ps[4 + q % 4][:, :, :], 1.0).then_inc(sem["act_sem"], 1)

        def s_out(s):
            for d_i in range(npos // 2):
                s.wait_ge(sem["act_sem"], 2 * d_i + 2)
                o = (2 * d_i) % NOUT
                dst = bass.AP(out_ext, 2 * d_i * 2 * P * DIM,
                              [[DIM, P], [2 * P * DIM, 2], [P * DIM, 2], [1, DIM]])
                s.dma_start(dst, outb[:, o:o + 2, :, :],
                            ).then_inc(sem[f"dma_s{d_i % NDMA_SEM}"], 16)
            for j in range(NDMA_SEM):
                uses = len([d for d in range(npos // 2) if d % NDMA_SEM == j])
                s.wait_ge(sem[f"dma_s{j}"], 16 * uses)
            for j in range(NSTG_SEM):
                uses = len([b for b in range(npos // SBATCH)
                            if b % NSTG_SEM == j])
                s.wait_ge(sem[f"stg_s{j}"], 16 * uses)

        with nc.Block() as block:
            @block.tensor
            def _(tensor):
                p_pos(tensor)

            @block.vector
            def _(vector):
                d_pos(vector)

            @block.scalar
            def _(scalar):
                a_in(scalar)
                a_pos(scalar)

            @block.gpsimd
            def _(gpsimd):
                g_stage(gpsimd)

            @block.sync
            def _(sync):
                s_in(sync)
                s_out(sync)

    return nc


def _expm(a):
    """expm via scaling-and-squaring Taylor in f64."""
    a = np.asarray(a, dtype=np.float64)
    nrm = np.linalg.norm(a, ord=1)
    s = max(0, int(np.ceil(np.log2(max(nrm, 1e-30) / 0.25))))
    a = a / (2.0 ** s)
    n = a.shape[0]
    e = np.eye(n)
    term = np.eye(n)
    for k in range(1, 20):
        term = term @ a / k
        e = e + term
        if np.abs(term).max() < 1e-18:
            break
    for _ in range(s):
        e = e @ e
    return e


def _tables(primitives):
    """R[q] (f64) for q in [1,128): R(1)=I, R(q) = M_{q&1} @ R(q>>1)."""
    prims = np.asarray(primitives, dtype=np.float64)
    mats = []
    for b in range(2):
        skew = prims[b] - prims[b].T
        mats.append(_expm(skew).T)         # M_b = P_b^T
    R = [None] * 128
    R[1] = np.eye(DIM)
    for q in range(2, 128):
        R[q] = mats[q & 1] @ R[q >> 1]
    return R


def _indices(u):
    """u: (n,) positions -> idxA (transposed tab), idxB, idxC (natural)."""
    u = u.astype(np.int64)
    blen = np.zeros_like(u)
    t = u.copy()
    while np.any(t > 0):
        blen = np.where(t > 0, blen + 1, blen)
        t >>= 1
    k = blen - 1
    tA = np.minimum(k, 6)
    idxA = (1 << tA) + (u & ((1 << tA) - 1))
    tB = np.clip(k - 6, 0, 5)
    idxB = (1 << tB) + ((u >> 6) & ((1 << tB) - 1))
    tC = np.clip(k - 11, 0, 5)
    idxC = (1 << tC) + ((u >> 11) & ((1 << tC) - 1))
    short = u < 64
    idxA = np.where(short, 1, idxA)
    idxB = np.where(short, u, idxB)
    assert idxA.max() < 128 and idxB.max() < 64 and idxC.max() < 64
    assert np.all((idxA == 1) | (idxA >= 64))
    return idxA, idxB, idxC


def kernel(primitives, identity, unique):
    global LAST_RESULTS
    from concourse.bass_utils import run_bass_kernel_spmd

    u = np.asarray(unique).astype(np.int64).ravel()
    n = u.shape[0]
    assert n % NCORES == 0
    npos = n // NCORES
    nsteps = npos + CLAG
    ngrp = (nsteps + 3) // 4
    ncolw = (ngrp + P - 1) // P

    R = _tables(primitives)
    # kc-split per-partition layouts: half h of entry M is M[h*128+p, j]
    rn0_np = np.zeros((P, NAT_E * HENT), np.float16)
    rn1_np = np.zeros((P, NAT_E * HENT), np.float16)
    for q in range(1, 64):
        m = R[q].astype(np.float16)
        rn0_np[:, (q - 1) * HENT:q * HENT] = m[0:P, :]
        rn1_np[:, (q - 1) * HENT:q * HENT] = m[P:2 * P, :]

    idxA, idxB, idxC = _indices(u)
    slotA = np.where(idxA == 1, 0, idxA - 63)
    vC_all = ((idxC - 1) * HENT).astype(np.int32)

    # rn chunk boundaries must match _build_nc
    ch = [(k * NAT_E) // RN_CHUNKS for k in range(RN_CHUNKS + 1)]
    cht = RT_CH_BOUNDS
    slotC = (idxC - 1).astype(np.int64)
    chunk_of = np.zeros(n, np.int64)
    for k in range(RN_CHUNKS):
        chunk_of[(slotC >= ch[k]) & (slotC < ch[k + 1])] = k

    # per-core sort by rn chunk; rn_need[q] = chunks resident before mm2
    # step q, max'd across cores (one SPMD program must fit all cores).
    # rt is stored per-core in FIRST-USE rank order so it can be loaded in
    # chunks gated by rt_need[s] while the pipeline runs.
    perms = []
    sorted_chunks = []
    rank_of_core = []           # per core: slotA -> rank (first-use order)
    rank_seq = []               # per core: per sorted step, the rank used
    for c in range(NCORES):
        sl = slice(c * npos, (c + 1) * npos)
        order = np.argsort(chunk_of[sl], kind="stable")
        perms.append(order)
        sorted_chunks.append(chunk_of[sl][order])
        sA = slotA[sl][order]
        rank = {}
        seq = np.zeros(npos, np.int64)
        for s in range(npos):
            v = int(sA[s])
            if v not in rank:
                rank[v] = len(rank)
            seq[s] = rank[v]
        for v in range(TRA_E):          # unused entries fill the tail ranks
            if v not in rank:
                rank[v] = len(rank)
        rank_of_core.append(rank)
        rank_seq.append(seq)
    rn_need = (np.max(np.stack(sorted_chunks), axis=0) + 1).astype(np.int64)
    rn_need = np.maximum.accumulate(rn_need)

    max_rank = np.maximum.accumulate(np.max(np.stack(rank_seq), axis=0))
    chunk_of_rank = np.zeros(TRA_E, np.int64)
    for k in range(RT_CHUNKS):
        chunk_of_rank[cht[k]:cht[k + 1]] = k
    rt_need = chunk_of_rank[max_rank] + 1
    key = (npos, tuple(rn_need.tolist()), tuple(rt_need.tolist()))
    if key not in _NC_CACHE:
        nc = _build_nc(npos, rn_need.tolist(), rt_need.tolist())
        nc.compile()
        _NC_CACHE[key] = nc
    nc = _NC_CACHE[key]

    in_maps = []
    for c in range(NCORES):
        sl = slice(c * npos, (c + 1) * npos)
        order = perms[c]
        iB = idxB[sl][order]
        a_v = (rank_seq[c] * HENT).astype(np.int32)
        c_v = vC_all[sl][order]

        # per-core rt tables in first-use rank order
        rt0_c = np.zeros((P, TRA_E * HENT), np.float16)
        rt1_c = np.zeros((P, TRA_E * HENT), np.float16)
        slot_to_q = {0: 1}
        for q in range(64, 128):
            slot_to_q[q - 63] = q
        for v, r in rank_of_core[c].items():
            m = R[slot_to_q[v]].T.astype(np.float16)
            rt0_c[:, r * HENT:(r + 1) * HENT] = m[0:P, :]
            rt1_c[:, r * HENT:(r + 1) * HENT] = m[P:2 * P, :]

        # bstream: entry idxB[k] in full [p, kc*256+j] layout, partition-major
        bstream = np.zeros((P, npos * ENT), np.float16)
        for k_i in range(npos):
            q = iB[k_i]
            m = R[q].astype(np.float16)
            bstream[:, k_i * ENT:k_i * ENT + HENT] = m[0:P, :]
            bstream[:, k_i * ENT + HENT:(k_i + 1) * ENT] = m[P:2 * P, :]

        offs_np = np.zeros((P, 8 * ncolw), np.int32)
        for g in range(ngrp):
            w = np.zeros(8, np.int32)
            for h in range(4):
                s = 4 * g + h
                if s < npos:
                    w[2 * h] = a_v[s]
                q = s - CLAG
                if 0 <= q < npos:
                    w[2 * h + 1] = c_v[q]
            offs_np[g % P, 8 * (g // P):8 * (g // P) + 8] = w

        in_maps.append({"rt0": rt0_c, "rt1": rt1_c, "rn0": rn0_np,
                        "rn1": rn1_np, "bstream": bstream, "offs": offs_np})

    import os
    trace_dir = os.environ.get("KERNEL_TRACE_DIR")
    res = run_bass_kernel_spmd(nc, in_maps, core_ids=list(range(NCORES)),
                               tmpdir=trace_dir)
    LAST_RESULTS = res

    out = np.empty((n, DIM, DIM), np.float32)
    for c in range(NCORES):
        o = np.asarray(res.results[c]["out"])  # (npos, 2, 128, 256) f16
        o = o.reshape(npos, DIM, DIM).astype(np.float32)
        out[c * npos + perms[c]] = o

    ident = np.asarray(identity, dtype=np.float32)[0]
    if not np.allclose(ident, np.eye(DIM, dtype=np.float32)):
        out = np.einsum("ij,njk->nik", ident, out).astype(np.float32)
    return out
